# revision 21
# baseline (speedup 1.0000x reference)
"""Trainium2 Bass kernel for nn_ARMAPosteriorModel.

The reference's windowed ARMA computation is mathematically a first-order
linear recurrence over time:

    a_prev[t] = sigmoid(a_raw)[t-1]        (a_prev[0] = 0)
    s[t]      = softplus(s_raw)[t]
    mean[t]   = (1 - a_prev[t]) * m[t]
    z[s,t]    = mean[t] + s[t] * noise[s,t]
    param[s,t] = a_prev[t] * param[s,t-1] + z[s,t]
    log_prob[s,t] = -log(s[t]) - 0.5*log(2*pi) - 0.5*noise[s,t]^2

The tiny (T,D,P) parameter tensors are preprocessed on host; the S-scaled
work (z, the scan, log_prob) runs on 8 NeuronCores, data-parallel over the
sample axis S (32 samples per core).  The time recurrence is computed with
the DVE `tensor_tensor_scan` instruction (one lane per (sample,dim,param)
stream, scan along the free/time axis).

Device layout per core: partition index = (s4, d, p) with s4 = sample%4,
free axis = t.  8 tile-groups of 4 samples each -> (8, 128, 1024) tensors.
"""

import sys

if "/opt/trn_rl_repo" not in sys.path:
    sys.path.insert(0, "/opt/trn_rl_repo")

import numpy as np

N_CORES = 8
S = 256
T = 1024
D = 4
P = 8
S_LOCAL = S // N_CORES           # 32 samples per core
G = S_LOCAL * D * P // 128       # 8 partition-groups of (4 samples x D x P)
GW = 2                           # partition-groups interleaved per SBUF tile
LOG2PI = float(np.log(2.0 * np.pi))

_NC_CACHE = {}


def _build_bass(dtype_name="float16"):
    """Build the Bass module (same NEFF for all cores; SPMD over in_maps)."""
    import concourse.tile as tile
    from concourse import bacc, mybir

    nc = bacc.Bacc(
        "TRN2",
        target_bir_lowering=False,
        debug=False,
        num_devices=N_CORES,
    )
    f32 = getattr(mybir.dt, dtype_name)

    noise_in = nc.dram_tensor(
        "noise", [G // GW, 128, GW * T], f32, kind="ExternalInput"
    )
    pp_in = nc.dram_tensor("pp", [4, 128, T], f32, kind="ExternalInput")
    param_out = nc.dram_tensor(
        "param", [G // GW, 128, GW * T], f32, kind="ExternalOutput"
    )
    lp_out = nc.dram_tensor("lp", [G // GW, 128, GW * T], f32, kind="ExternalOutput")

    W = GW * T       # free width per tile (GW groups side by side)

    with tile.TileContext(nc) as tc:
        with (
            tc.tile_pool(name="const", bufs=1) as cpool,
            tc.tile_pool(name="nin", bufs=3) as npool,
            tc.tile_pool(name="work", bufs=2) as wpool,
            tc.tile_pool(name="outp", bufs=2) as opool,
        ):
            # Parameter tensors, host-replicated to 128 partitions;
            # duplicated GW-fold along the free axis so a (128, W) op can
            # read them with a plain contiguous AP.
            consts = []
            for idx in range(4):
                ct = cpool.tile([128, W], f32, tag=f"c{idx}", name=f"c{idx}")
                for j in range(GW):
                    nc.sync.dma_start(ct[:, j * T : (j + 1) * T], pp_in[idx])
                consts.append(ct)
            A, SC, MU, NL = consts

            for g in range(G // GW):
                nt = npool.tile([128, W], f32, tag="noise", name=f"nt{g}")
                nc.sync.dma_start(nt[:], noise_in[g])

                # log_prob path: NL - (noise/sqrt(2))^2
                sq = wpool.tile([128, W], f32, tag="sq", name=f"sq{g}")
                nc.scalar.activation(
                    sq[:],
                    nt[:],
                    mybir.ActivationFunctionType.Square,
                    scale=0.7071067811865476,
                )
                lt = opool.tile([128, W], f32, tag="lp", name=f"lt{g}")
                nc.vector.tensor_sub(lt[:], NL[:], sq[:])
                nc.sync.dma_start(lp_out[g], lt[:])

                # param path: scan(A, mean + s*noise); a_prev[0] == 0 resets
                # the recurrence at each group boundary inside the tile.
                sn = wpool.tile([128, W], f32, tag="sn", name=f"sn{g}")
                nc.vector.tensor_mul(sn[:], nt[:], SC[:])
                z = wpool.tile([128, W], f32, tag="z", name=f"z{g}")
                nc.vector.tensor_add(z[:], sn[:], MU[:])
                pt = opool.tile([128, W], f32, tag="param", name=f"pt{g}")
                nc.vector.tensor_tensor_scan(
                    pt[:],
                    A[:],
                    z[:],
                    0.0,
                    mybir.AluOpType.mult,
                    mybir.AluOpType.add,
                )
                nc.sync.dma_start(param_out[g], pt[:])
    nc.finalize()
    return nc


DTYPE = "float16"          # device storage/compute dtype ("float16"|"float32")


def _get_nc():
    if DTYPE not in _NC_CACHE:
        _NC_CACHE[DTYPE] = _build_bass(DTYPE)
    return _NC_CACHE[DTYPE]


def _host_prep(m, s_raw, a_raw, dim_idx):
    """Precompute the small per-(t,d,p) parameter tensors -> pp (4, 32, T)."""
    mm = np.asarray(m)[:, dim_idx].astype(np.float64)          # (T, D, P)
    sr = np.asarray(s_raw)[:, dim_idx].astype(np.float64)      # (T, D, P)
    ar = np.asarray(a_raw)[:, dim_idx, 0].astype(np.float64)   # (T-1, D)

    s = np.logaddexp(0.0, sr)                                  # softplus
    a = 1.0 / (1.0 + np.exp(-ar))                              # sigmoid
    a_prev = np.zeros((T, D))
    a_prev[1:] = a
    mean = (1.0 - a_prev)[:, :, None] * mm                     # (T, D, P)
    nlogs = -np.log(s) - 0.5 * LOG2PI                          # (T, D, P)

    # row = d*P + p, col = t; tiled 4x along partitions (s4 replicas)
    A_small = np.broadcast_to(a_prev.T[:, None, :], (D, P, T)).reshape(D * P, T)
    S_small = s.transpose(1, 2, 0).reshape(D * P, T)
    M_small = mean.transpose(1, 2, 0).reshape(D * P, T)
    N_small = nlogs.transpose(1, 2, 0).reshape(D * P, T)
    small = np.stack([A_small, S_small, M_small, N_small])     # (4, 32, T)
    np_dt = np.float16 if DTYPE == "float16" else np.float32
    return np.ascontiguousarray(np.tile(small, (1, 4, 1)).astype(np_dt))


def _run_device(noise_r, pp, trace=False):
    """noise_r: (S, D, P, T) device-dtype contiguous. Returns (param, lp) as
    (S, D, P, T) device-dtype plus the BassKernelResults."""
    from concourse.bass_utils import run_bass_kernel_spmd

    nc = _get_nc()
    in_maps = []
    for c in range(N_CORES):
        shard = noise_r[S_LOCAL * c : S_LOCAL * (c + 1)].reshape(
            G // GW, GW, 128, T
        )
        shard = np.ascontiguousarray(shard.transpose(0, 2, 1, 3)).reshape(
            G // GW, 128, GW * T
        )
        in_maps.append({"noise": shard, "pp": pp})

    kw = {}
    if trace:
        kw = dict(trace=True, trace_cores=list(range(N_CORES)))
    res = run_bass_kernel_spmd(nc, in_maps, core_ids=list(range(N_CORES)), **kw)

    np_dt = np.float16 if DTYPE == "float16" else np.float32
    param = np.empty((S, D, P, T), np_dt)
    lp = np.empty((S, D, P, T), np_dt)

    def _deinterleave(x):
        # (G//GW, 128, GW*T) -> (G, 128, T) -> (S_LOCAL, D, P, T)
        x = x.reshape(G // GW, 128, GW, T).transpose(0, 2, 1, 3)
        return x.reshape(S_LOCAL, D, P, T)

    for c in range(N_CORES):
        out = res.results[c]
        param[S_LOCAL * c : S_LOCAL * (c + 1)] = _deinterleave(out["param"])
        lp[S_LOCAL * c : S_LOCAL * (c + 1)] = _deinterleave(out["lp"])
    return param, lp, res


def kernel(
    y=None,
    age=None,
    m=None,
    s_raw=None,
    a_raw=None,
    noise=None,
    cond_sample=None,
    dim_idx=None,
    compute_log_prob=1,
    _trace=False,
    **_unused,
):
    np_dt = np.float16 if DTYPE == "float16" else np.float32
    noise = np.asarray(noise)
    dim_idx = np.asarray(dim_idx)
    pp = _host_prep(m, s_raw, a_raw, dim_idx)

    # (S, T, D, P) -> (S, D, P, T) so the time axis is contiguous.
    noise_r = np.ascontiguousarray(noise.transpose(0, 2, 3, 1).astype(np_dt))

    param_r, lp_r, res = _run_device(noise_r, pp, trace=_trace)

    param = np.ascontiguousarray(param_r.transpose(0, 3, 1, 2).astype(np.float32))
    lp = np.ascontiguousarray(lp_r.transpose(0, 3, 1, 2).astype(np.float32))
    kernel.last_results = res
    if compute_log_prob:
        return (param, lp)
    return param
